# revision 21
# baseline (speedup 1.0000x reference)
"""Multi-head attention forward on 8 Trainium2 NeuronCores.

Problem: x [2,2048,1024], weights wq/wk/wv/wo [1024,1024] (torch Linear
layout, y = x @ W.T), 16 heads, head_dim 64, fp32.

Sharding: core c handles batch b = c//4 and head group g = c%4 (heads
4g..4g+3, i.e. 256 output dims of wq/wk/wv and 256 input dims of wo).
Each core computes a partial output [2048, 1024]; the host sums the 4
partials per batch (the reduce is host-side, no collectives).

On-core plan (v2 — tuned for the HAM activity throttle, which caps
sustained PE activity at ~0.65 of full clock; total PE streamed columns
is the budget that matters):
  Load: x and wq/wk/wv are host-cast to bf16 (halves DMA bytes, no
  on-chip casts); wo arrives f32 via gpsimd casting DMA -> f32r.
  Phase 1 projections overlap the x DMA: q runs k-tile-major (both
  m-tiles resident in PSUM, 8 banks) consuming x k-tiles as they land,
  then k-proj, then v. All projection operands bf16 (1 cyc/col).
  Attention per head-pair as before: two heads' j-streams interleaved,
  scores DEPTH=6 ahead of AV; exp entirely on ACT (the HAM cap slows
  the PE below the ACT rate, so no DVE offload is needed).
  Normalization is pair-packed: head A's o stays on partitions 0-63
  (DVE copy), head B's o is DMA-shifted to partitions 64-127, colsums
  broadcast via two PE outer products into one [128, IB] reciprocal,
  one tensor_mul -> o_sb [128 (pair), 2 groups, NB, IB].
  Output projection contracts over 128 partitions (two heads at once):
  half the streamed columns of the per-head version.
"""

import numpy as np
from contextlib import ExitStack

import ml_dtypes

import concourse.bacc as bacc
import concourse.bass as bass
import concourse.mybir as mybir
import concourse.tile as tile
from concourse.bass_utils import run_bass_kernel_spmd

f32 = mybir.dt.float32
f32r = mybir.dt.float32r
bf16 = mybir.dt.bfloat16
i32 = mybir.dt.int32
EXP = mybir.ActivationFunctionType.Exp

# ---- custom DVE op: exp correction multiply --------------------------------
# Schraudolph-style exp on DVE (3 passes, offloads part of softmax's exp from
# the ACT engine so the PE never stalls waiting for probabilities):
#   p1 (std):  u = int32(score * A + B)     A = 0.125*log2(e)*2^23, B = 127*2^23
#              => bitcast(u) = S = 2^i*(1+f) with i+f = score*0.125*log2(e)
#   p2 (std):  r = (u & 0x7FFFFF) | 0x3F800000        => r = 1+f in [1,2)
#   p3 (cust): out = S * (q0 + r*(q1 + r*q2))  ~= S * 2^f/(1+f) = exp(score/8)
# Correction quadratic fit minimax on [1,2]: rel err <= 6.6e-3, unbiased.
EXP_A = float(0.125 * np.log2(np.e) * 2**23)
EXP_B = float(127 * 2**23)
EXP_MASK = 0x007FFFFF
EXP_OR = 0x3F800000
EXP_Q0 = 1.43400066
EXP_Q1 = -0.66623009
EXP_Q2 = 0.22566318

_EXP_CORR = None


def _ensure_exp_corr():
    global _EXP_CORR
    if _EXP_CORR is not None:
        return _EXP_CORR
    import concourse.dve_ops as dve_ops
    from concourse.dve_spec import Spec, Src0, Src1, C0, C1, C2

    def _ref(in0, in1, c0, c1, c2):
        return in1 * (c2 + in0 * (c0 + in0 * c1))

    op = dve_ops.DveOp(
        "EXP_CORR_ANT",
        Spec(body=Src1 * (C2 + Src0 * (C0 + Src0 * C1)), reference=_ref),
        subdim=False,
        uops_sha={},
    )
    if op.name not in dve_ops._SUB_OPCODE_FOR_NAME:
        dve_ops.OPS.append(op)
        dve_ops.CUSTOM_DVE_SPECS[op.name] = op.spec
        dve_ops._SUB_OPCODE_FOR_NAME[op.name] = (
            max(dve_ops._SUB_OPCODE_FOR_NAME.values()) + 1
        )
    # pin the uops sha (first compile reports the computed value)
    for ver in ("v3",):
        try:
            op.compile(ver)
        except ValueError as e:
            msg = str(e)
            got = msg.split(f"{ver}: ")[1].split(" ")[0]
            op.uops_sha[ver] = got
            op.compile(ver)
    _EXP_CORR = op
    return _EXP_CORR

B, S, D = 2, 2048, 1024
H, DH = 16, 64
NCORES = 8
GROUPS = NCORES // B           # 4 head-groups per batch
HPC = H // GROUPS              # 4 heads per core
DLOC = HPC * DH                # 256
KT = D // 128                  # 8 contraction tiles
ST = S // 128                  # 16 sequence tiles
NB = 2                         # i-blocks
IB = S // NB                   # 1024
NCH = IB // 512                # 512-wide matmul chunks per i-block
NPAIR = HPC // 2               # 2 head pair-groups per core


def _emit(tc, nc):
    # wq/wk/wv arrive pre-interleaved to the SBUF layout: row p holds
    # concat_k w.T[k*128+p, :], so the DMA lines are contiguous per
    # partition (2KB per half) instead of 512B strided.
    xT = nc.dram_tensor("xTb", [D, S], bf16, kind="ExternalInput").ap()
    wqT = nc.dram_tensor("wqb", [128, KT * DLOC], bf16, kind="ExternalInput").ap()
    wkT = nc.dram_tensor("wkb", [128, KT * DLOC], bf16, kind="ExternalInput").ap()
    wvT = nc.dram_tensor("wvb", [128, KT * DLOC], bf16, kind="ExternalInput").ap()
    woT = nc.dram_tensor("woT", [DLOC, D], f32, kind="ExternalInput").ap()
    outp = nc.dram_tensor("outp", [S, D], f32, kind="ExternalOutput").ap()

    with ExitStack() as ctx:
        wpool = ctx.enter_context(tc.tile_pool(name="wpool", bufs=1))
        qkv = ctx.enter_context(tc.tile_pool(name="qkv", bufs=1))
        small = ctx.enter_context(tc.tile_pool(name="smalls", bufs=2))

        # ---- constants ----
        ones_f = small.tile([128, HPC], f32, bufs=1)
        nc.vector.memset(ones_f, 1.0)
        ones65f = small.tile([65, 64], f32, bufs=1)
        nc.vector.memset(ones65f, 1.0)
        ones65 = small.tile([65, 64], f32r, bufs=1)
        nc.vector.tensor_copy(ones65, ones65f)

        # persistent attention operands
        qt = qkv.tile([128, 2, S], bf16)
        kt = qkv.tile([128, 2, S], bf16)
        v_sb = qkv.tile([128, ST, HPC, 65], bf16)
        # wo [256,1024] -> [128 part (pair-packed: h%2*64+d), 2 groups, 1024]
        wo_r = wpool.tile([128, NPAIR, D], f32r)
        wov = woT.rearrange("(g p) e -> p g e", p=128)
        for g in range(NPAIR):
            nc.gpsimd.dma_start(out=wo_r[:, g], in_=wov[:, g])

        # ---- phase 1: load + projections, overlapped ----
        # Weights arrive host-interleaved ([128 part, k, m] contiguous per
        # partition -> 2KB DMA lines instead of 512B) and split in k-halves
        # so the first q matmuls unblock as early as possible. q runs
        # k-tile-major (4 PSUM quarter-tiles resident, 8 banks) consuming x
        # k-tiles as they land, then v. The k projection is NOT emitted
        # here: it runs as filler work inside the first attention pair.
        ph1_cm = tc.tile_pool(name="ph1", bufs=1)
        ph1 = ph1_cm.__enter__()
        x_sb = ph1.tile([128, KT, S], bf16)
        wq_r = ph1.tile([128, KT, DLOC], bf16, name="wq_r")
        wk_r = ph1.tile([128, KT, DLOC], bf16, name="wk_r")
        wv_r = ph1.tile([128, KT, DLOC], bf16, name="wv_r")

        xv = xT.rearrange("(k p) s -> p k s", p=128)
        wqv = wqT.rearrange("p (k m) -> p k m", k=KT)
        wkv = wkT.rearrange("p (k m) -> p k m", k=KT)
        wvv = wvT.rearrange("p (k m) -> p k m", k=KT)
        KH = KT // 2
        # Queue plan (q consumes x k-tiles as they land, ~1.7us apart):
        #   sync:   x0a x0b x2 x4 x6 wv0 wv1 wk1
        #   scalar: wq0 x1 wq1 x3 x5 x7 wk0
        nc.sync.dma_start(out=x_sb[:, 0, 0:IB], in_=xv[:, 0, 0:IB])
        nc.sync.dma_start(out=x_sb[:, 0, IB:S], in_=xv[:, 0, IB:S])
        nc.scalar.dma_start(out=wq_r[:, 0:KH], in_=wqv[:, 0:KH])
        nc.scalar.dma_start(out=x_sb[:, 1], in_=xv[:, 1])
        nc.scalar.dma_start(out=wq_r[:, KH:KT], in_=wqv[:, KH:KT])
        for k in range(2, KT, 2):
            nc.sync.dma_start(out=x_sb[:, k], in_=xv[:, k])
        for k in range(3, KT, 2):
            nc.scalar.dma_start(out=x_sb[:, k], in_=xv[:, k])
        for h in range(2):
            nc.sync.dma_start(
                out=wv_r[:, h * KH : (h + 1) * KH],
                in_=wvv[:, h * KH : (h + 1) * KH],
            )
        nc.scalar.dma_start(out=wk_r[:, 0:KH], in_=wkv[:, 0:KH])
        nc.sync.dma_start(out=wk_r[:, KH:KT], in_=wkv[:, KH:KT])

        with tc.tile_pool(name="psq", bufs=4, space="PSUM") as psq:
            # q k-major: 4 quarter-tiles [128, 1024] (8 banks) so the
            # PSUM->SBUF copies drain per-quarter.
            pq = [
                psq.tile([128, IB], f32, tag="pq", name=f"pq{i}")
                for i in range(4)
            ]
            for k in range(KT):
                for m in range(2):
                    for half in range(2):
                        for ch in range(NCH):
                            nc.tensor.matmul(
                                pq[m * 2 + half][:, ch * 512 : (ch + 1) * 512],
                                lhsT=wq_r[:, k, m * 128 : (m + 1) * 128],
                                rhs=x_sb[
                                    :,
                                    k,
                                    half * IB + ch * 512 : half * IB + (ch + 1) * 512,
                                ],
                                start=(k == 0),
                                stop=(k == KT - 1),
                            )
            for m in range(2):
                for half in range(2):
                    nc.vector.tensor_copy(
                        qt[:, m, half * IB : (half + 1) * IB], pq[m * 2 + half]
                    )

        with tc.tile_pool(name="psv", bufs=4, space="PSUM") as psv:
            for st_i in range(ST):
                pv = psv.tile([128, DLOC], f32, tag="pv", name="pv")
                for k in range(KT):
                    nc.tensor.matmul(
                        pv,
                        lhsT=x_sb[:, k, st_i * 128 : (st_i + 1) * 128],
                        rhs=wv_r[:, k],
                        start=(k == 0),
                        stop=(k == KT - 1),
                    )
                nc.vector.tensor_copy(
                    v_sb[:, st_i, :, 0:64],
                    pv.rearrange("p (h d) -> p h d", h=HPC),
                )
                nc.vector.tensor_copy(v_sb[:, st_i, :, 64], ones_f)

        with tc.tile_pool(name="psk", bufs=4, space="PSUM") as psk:
            pk = [
                psk.tile([128, IB], f32, tag="pk", name=f"pk{i}")
                for i in range(4)
            ]
            for k in range(KT):
                for m in range(2):
                    for half in range(2):
                        for ch in range(NCH):
                            nc.tensor.matmul(
                                pk[m * 2 + half][:, ch * 512 : (ch + 1) * 512],
                                lhsT=wk_r[:, k, m * 128 : (m + 1) * 128],
                                rhs=x_sb[
                                    :,
                                    k,
                                    half * IB + ch * 512 : half * IB + (ch + 1) * 512,
                                ],
                                start=(k == 0),
                                stop=(k == KT - 1),
                            )
            for m in range(2):
                for half in range(2):
                    nc.vector.tensor_copy(
                        kt[:, m, half * IB : (half + 1) * IB], pk[m * 2 + half]
                    )

        ph1_cm.__exit__(None, None, None)

        # ---- attention pools (allocated after phase-1 space released) ----
        ptp = ctx.enter_context(tc.tile_pool(name="ptp", bufs=8))
        osb = ctx.enter_context(tc.tile_pool(name="osb", bufs=1))
        outsb = ctx.enter_context(tc.tile_pool(name="outsb", bufs=3))
        norm = ctx.enter_context(tc.tile_pool(name="norm", bufs=2))
        ps = ctx.enter_context(tc.tile_pool(name="ps", bufs=2, space="PSUM"))
        pso = ctx.enter_context(tc.tile_pool(name="pso", bufs=2, space="PSUM"))
        # normalized o^T, pair-packed: [128 = (h%2)*64+d, pair group, ib, i]
        o_sb = osb.tile([128, NPAIR, NB, IB], f32r, name="o_sb")

        exp_corr = _ensure_exp_corr()
        alu = bass.mybir.AluOpType
        DVE_JTS = frozenset((1, 5, 9, 13))  # 4/16 exps offloaded to DVE

        def emit_head_pair(ib, h0, extra=None, fast_shift=False):
            """Attention for heads (h0, h0+1): the two heads' j-streams are
            interleaved so the PE always has the other head's matmuls while an
            exp is in flight. `extra` is a list of closures emitting filler PE
            work (deferred output-projection blocks), spread over the loop."""
            heads = (h0, h0 + 1)
            o_augs = {}
            for h in heads:
                o_augs[h] = pso.tile([65, IB], f32, tag="pso", name="o_aug")

            def scores(h, jt):
                p0 = (h % 2) * 64
                mi = h // 2
                ssc = ps.tile([128, IB], f32, tag="ps", name="ssc")
                for ch in range(NCH):
                    nc.tensor.matmul(
                        ssc[:, ch * 512 : (ch + 1) * 512],
                        lhsT=kt[p0 : p0 + 64, mi, jt * 128 : (jt + 1) * 128],
                        rhs=qt[
                            p0 : p0 + 64,
                            mi,
                            ib * IB + ch * 512 : ib * IB + (ch + 1) * 512,
                        ],
                        start=True,
                        stop=True,
                    )
                pt = ptp.tile([128, IB], bf16, tag="pt", name="pt")
                if jt in DVE_JTS:
                    ue = ptp.tile([128, IB], i32, tag="ue", name="ue", bufs=4)
                    nc.vector.tensor_scalar(
                        ue, ssc, EXP_A, EXP_B, alu.mult, alu.add
                    )
                    re = ptp.tile([128, IB], i32, tag="re", name="re", bufs=4)
                    nc.vector.tensor_scalar(
                        re, ue, EXP_MASK, EXP_OR, alu.bitwise_and, alu.bitwise_or
                    )
                    nc.vector._custom_dve(
                        exp_corr,
                        out=pt,
                        in0=re.bitcast(f32),
                        in1=ue.bitcast(f32),
                        s0=EXP_Q1,
                        s1=EXP_Q2,
                        imm2=EXP_Q0,
                    )
                else:
                    nc.scalar.activation(pt, ssc, EXP, scale=0.125)
                return pt

            def av(h, jt, pt):
                for ch in range(NCH):
                    nc.tensor.matmul(
                        o_augs[h][:, ch * 512 : (ch + 1) * 512],
                        lhsT=v_sb[:, jt, h, :],
                        rhs=pt[:, ch * 512 : (ch + 1) * 512],
                        start=(jt == 0),
                        stop=(jt == ST - 1),
                    )

            order = [(h, jt) for jt in range(ST) for h in heads]
            DEPTH = 6
            extra = list(extra or [])
            pts = {}
            for n, (h, jt) in enumerate(order):
                pts[(h, jt)] = scores(h, jt)
                if n >= DEPTH:
                    key = order[n - DEPTH]
                    av(*key, pts.pop(key))
                if extra and n % 3 == 2:
                    extra.pop(0)()
            for key in order[-DEPTH:]:
                av(*key, pts.pop(key))
            for fn in extra:
                fn()

            # Normalize both heads (baseline scheme: copy o_aug to SBUF,
            # broadcast the colsum via PE outer product, multiply by its
            # reciprocal). Even heads write o_sb rows 0-63 directly; odd
            # heads are normalized into a temp and DMA-shifted to rows
            # 64-127 (engines can't cross partitions, DMA can) so the
            # output projection can contract over all 128 partitions.
            # The odd head goes first so its shift DMA overlaps the even
            # head's norm; `fast_shift` routes it via a HWDGE queue (the
            # kernel tail waits on this DMA for the last pair).
            g = h0 // 2
            for h in (h0 + 1, h0):
                o_aug = o_augs[h]
                o_cp = norm.tile([65, IB], f32r, tag="ocp", name="o_cp")
                nc.vector.tensor_copy(o_cp, o_aug)
                cb_ps = pso.tile([65, IB], f32, tag="pso", name="cb_ps")
                for ch in range(NCH):
                    nc.tensor.matmul(
                        cb_ps[0:64, ch * 512 : (ch + 1) * 512],
                        lhsT=ones65[64:65, :],
                        rhs=o_cp[64:65, ch * 512 : (ch + 1) * 512],
                        start=True,
                        stop=True,
                    )
                rb_f = norm.tile([64, IB], f32, tag="rb_f", name="rb_f")
                nc.vector.reciprocal_approx_fast(rb_f, cb_ps[0:64, :])
                if h % 2 == 0:
                    nc.vector.tensor_mul(
                        o_sb[0:64, g, ib], o_cp[0:64, :], rb_f
                    )
                else:
                    on_t = norm.tile([64, IB], f32r, tag="on_t", name="on_t")
                    nc.vector.tensor_mul(on_t, o_cp[0:64, :], rb_f)
                    eng = nc.sync if fast_shift else nc.gpsimd
                    eng.dma_start(out=o_sb[64:128, g, ib], in_=on_t)

        # Output projection, pair-packed: contraction over 128 partitions
        # (two heads at once), 2 group-accumulated matmul pairs per i-tile.
        dacc = ctx.enter_context(tc.tile_pool(name="dacc", bufs=1))

        def emit_out_block(ib, it):
            """Full output projection for rows [ib*IB + it*128, +128)."""
            po = ps.tile([128, D], f32, tag="ps", name="po")
            for g in range(NPAIR):
                for ch in range(2):
                    nc.tensor.matmul(
                        po[:, ch * 512 : (ch + 1) * 512],
                        lhsT=o_sb[:, g, ib, it * 128 : (it + 1) * 128],
                        rhs=wo_r[:, g, ch * 512 : (ch + 1) * 512],
                        start=(g == 0),
                        stop=(g == NPAIR - 1),
                    )
            ot = outsb.tile([128, D], f32, tag="ot", name="ot")
            nc.vector.tensor_copy(ot, po)
            row = ib * IB + it * 128
            eng = nc.sync if it % 2 == 0 else nc.scalar
            eng.dma_start(out=outp[row : row + 128, :], in_=ot)

        def emit_d_g0(acc, ib, it):
            """Pair-group 0 half, accumulated in SBUF (filler work)."""
            po = ps.tile([128, D], f32, tag="ps", name="po")
            for ch in range(2):
                nc.tensor.matmul(
                    po[:, ch * 512 : (ch + 1) * 512],
                    lhsT=o_sb[:, 0, ib, it * 128 : (it + 1) * 128],
                    rhs=wo_r[:, 0, ch * 512 : (ch + 1) * 512],
                    start=True,
                    stop=True,
                )
            nc.vector.tensor_copy(acc[:, it], po)

        def emit_d_g1(acc, ib, it):
            """Pair-group 1 plus the accumulated group-0 half, then store."""
            po = ps.tile([128, D], f32, tag="ps", name="po")
            for ch in range(2):
                nc.tensor.matmul(
                    po[:, ch * 512 : (ch + 1) * 512],
                    lhsT=o_sb[:, 1, ib, it * 128 : (it + 1) * 128],
                    rhs=wo_r[:, 1, ch * 512 : (ch + 1) * 512],
                    start=True,
                    stop=True,
                )
            ot = outsb.tile([128, D], f32, tag="ot", name="ot")
            nc.vector.tensor_add(ot, po, acc[:, it])
            row = ib * IB + it * 128
            eng = nc.sync if it % 2 == 0 else nc.scalar
            eng.dma_start(out=outp[row : row + 128, :], in_=ot)

        # Schedule: i-block 0 attention plain, then i-block 1 with i-block
        # 0's full output projection in its first pair and i-block 1's
        # group-0 half in its second; the tail finishes group 1.
        d1_acc = dacc.tile([128, 8, D], f32, tag="dacc", name="d1_acc")
        emit_head_pair(0, 0)
        emit_head_pair(0, 2)
        emit_head_pair(
            1, 0, extra=[lambda it=it: emit_out_block(0, it) for it in range(8)]
        )
        emit_head_pair(
            1,
            2,
            extra=[lambda it=it: emit_d_g0(d1_acc, 1, it) for it in range(8)],
            fast_shift=True,
        )
        for it in range(8):
            emit_d_g1(d1_acc, 1, it)


_PROGRAM = None


def _program():
    global _PROGRAM
    if _PROGRAM is None:
        nc = bacc.Bacc("TRN2", target_bir_lowering=False, debug=False)
        with tile.TileContext(nc) as tc:
            _emit(tc, nc)
        nc.compile()
        _PROGRAM = nc
    return _PROGRAM


def _in_maps(x, wq, wk, wv, wo):
    bf = ml_dtypes.bfloat16

    def interleave(wT):
        # [D, DLOC] -> [128, KT*DLOC]: row p = concat_k wT[k*128+p, :]
        return np.ascontiguousarray(
            wT.reshape(KT, 128, DLOC).transpose(1, 0, 2).reshape(128, KT * DLOC)
        )

    maps = []
    for c in range(NCORES):
        b, g = divmod(c, GROUPS)
        rows = slice(g * DLOC, (g + 1) * DLOC)
        maps.append(
            {
                "xTb": np.ascontiguousarray(x[b].T.astype(bf)),
                "wqb": interleave(wq[rows, :].T.astype(bf)),
                "wkb": interleave(wk[rows, :].T.astype(bf)),
                "wvb": interleave(wv[rows, :].T.astype(bf)),
                "woT": np.ascontiguousarray(wo[:, rows].T),
            }
        )
    return maps


def kernel(x, e, wq, wk, wv, wo, **_unused):
    x = np.asarray(x, dtype=np.float32)
    wq = np.asarray(wq, dtype=np.float32)
    wk = np.asarray(wk, dtype=np.float32)
    wv = np.asarray(wv, dtype=np.float32)
    wo = np.asarray(wo, dtype=np.float32)

    nc = _program()
    in_maps = _in_maps(x, wq, wk, wv, wo)

    # Transient device corruption has been observed on this fabric
    # (NRT_EXEC_UNIT_UNRECOVERABLE events); sanity-check the partials and
    # retry up to twice if a core returned garbage (NaN/Inf, absurd
    # magnitudes, or an all-zero row block from a dropped DMA).
    def _sane(parts):
        for p in parts:
            if not np.isfinite(p).all():
                return False
            amax = np.abs(p).max()
            if amax > 1e6 or amax == 0.0:
                return False
            if (np.abs(p).max(axis=1) == 0.0).any():
                return False
        return True

    for _attempt in range(3):
        res = run_bass_kernel_spmd(nc, in_maps, list(range(NCORES))).results
        parts = [res[c]["outp"] for c in range(NCORES)]
        if _sane(parts):
            break

    out = np.empty((B, S, D), dtype=np.float32)
    for b in range(B):
        acc = parts[b * GROUPS].astype(np.float32)
        for g in range(1, GROUPS):
            acc = acc + parts[b * GROUPS + g]
        out[b] = acc
    return out


# revision 23
# speedup vs baseline: 1.1590x; 1.1590x over previous
"""Multi-head attention forward on 8 Trainium2 NeuronCores.

Problem: x [2,2048,1024], weights wq/wk/wv/wo [1024,1024] (torch Linear
layout, y = x @ W.T), 16 heads, head_dim 64, fp32.

Sharding: core c handles batch b = c//4 and head group g = c%4 (heads
4g..4g+3, i.e. 256 output dims of wq/wk/wv and 256 input dims of wo).
Each core computes a partial output [2048, 1024]; the host sums the 4
partials per batch (the reduce is host-side, no collectives).

On-core plan (v2 — tuned for the HAM activity throttle, which caps
sustained PE activity at ~0.65 of full clock; total PE streamed columns
is the budget that matters):
  Load: x and wq/wk/wv are host-cast to bf16 (halves DMA bytes, no
  on-chip casts); wo arrives f32 via gpsimd casting DMA -> f32r.
  Phase 1 projections overlap the x DMA: q runs k-tile-major (both
  m-tiles resident in PSUM, 8 banks) consuming x k-tiles as they land,
  then k-proj, then v. All projection operands bf16 (1 cyc/col).
  Attention per head-pair as before: two heads' j-streams interleaved,
  scores DEPTH=6 ahead of AV; exp entirely on ACT (the HAM cap slows
  the PE below the ACT rate, so no DVE offload is needed).
  Normalization is pair-packed: head A's o stays on partitions 0-63
  (DVE copy), head B's o is DMA-shifted to partitions 64-127, colsums
  broadcast via two PE outer products into one [128, IB] reciprocal,
  one tensor_mul -> o_sb [128 (pair), 2 groups, NB, IB].
  Output projection contracts over 128 partitions (two heads at once):
  half the streamed columns of the per-head version.
"""

import numpy as np
from contextlib import ExitStack

import ml_dtypes

import concourse.bacc as bacc
import concourse.bass as bass
import concourse.mybir as mybir
import concourse.tile as tile
from concourse.bass_utils import run_bass_kernel_spmd

f32 = mybir.dt.float32
f32r = mybir.dt.float32r
bf16 = mybir.dt.bfloat16
i32 = mybir.dt.int32
EXP = mybir.ActivationFunctionType.Exp

# ---- custom DVE op: exp correction multiply --------------------------------
# Schraudolph-style exp on DVE (3 passes, offloads part of softmax's exp from
# the ACT engine so the PE never stalls waiting for probabilities):
#   p1 (std):  u = int32(score * A + B)     A = 0.125*log2(e)*2^23, B = 127*2^23
#              => bitcast(u) = S = 2^i*(1+f) with i+f = score*0.125*log2(e)
#   p2 (std):  r = (u & 0x7FFFFF) | 0x3F800000        => r = 1+f in [1,2)
#   p3 (cust): out = S * (q0 + r*(q1 + r*q2))  ~= S * 2^f/(1+f) = exp(score/8)
# Correction quadratic fit minimax on [1,2]: rel err <= 6.6e-3, unbiased.
EXP_A = float(0.125 * np.log2(np.e) * 2**23)
EXP_B = float(127 * 2**23)
EXP_MASK = 0x007FFFFF
EXP_OR = 0x3F800000
EXP_Q0 = 1.43400066
EXP_Q1 = -0.66623009
EXP_Q2 = 0.22566318

_EXP_CORR = None


def _ensure_exp_corr():
    global _EXP_CORR
    if _EXP_CORR is not None:
        return _EXP_CORR
    import concourse.dve_ops as dve_ops
    from concourse.dve_spec import Spec, Src0, Src1, C0, C1, C2

    def _ref(in0, in1, c0, c1, c2):
        return in1 * (c2 + in0 * (c0 + in0 * c1))

    op = dve_ops.DveOp(
        "EXP_CORR_ANT",
        Spec(body=Src1 * (C2 + Src0 * (C0 + Src0 * C1)), reference=_ref),
        subdim=False,
        uops_sha={},
    )
    if op.name not in dve_ops._SUB_OPCODE_FOR_NAME:
        dve_ops.OPS.append(op)
        dve_ops.CUSTOM_DVE_SPECS[op.name] = op.spec
        dve_ops._SUB_OPCODE_FOR_NAME[op.name] = (
            max(dve_ops._SUB_OPCODE_FOR_NAME.values()) + 1
        )
    # pin the uops sha (first compile reports the computed value)
    for ver in ("v3",):
        try:
            op.compile(ver)
        except ValueError as e:
            msg = str(e)
            got = msg.split(f"{ver}: ")[1].split(" ")[0]
            op.uops_sha[ver] = got
            op.compile(ver)
    _EXP_CORR = op
    return _EXP_CORR

B, S, D = 2, 2048, 1024
H, DH = 16, 64
NCORES = 8
GROUPS = NCORES // B           # 4 head-groups per batch
HPC = H // GROUPS              # 4 heads per core
DLOC = HPC * DH                # 256
KT = D // 128                  # 8 contraction tiles
ST = S // 128                  # 16 sequence tiles
NB = 2                         # i-blocks
IB = S // NB                   # 1024
NCH = IB // 512                # 512-wide matmul chunks per i-block
NPAIR = HPC // 2               # 2 head pair-groups per core


def _emit(tc, nc):
    # wq/wk/wv arrive pre-interleaved to the SBUF layout: row p holds
    # concat_k w.T[k*128+p, :], so the DMA lines are contiguous per
    # partition (2KB per half) instead of 512B strided.
    xT = nc.dram_tensor("xTb", [D, S], bf16, kind="ExternalInput").ap()
    wqT = nc.dram_tensor("wqb", [128, KT * DLOC], bf16, kind="ExternalInput").ap()
    wkT = nc.dram_tensor("wkb", [128, KT * DLOC], bf16, kind="ExternalInput").ap()
    wvT = nc.dram_tensor("wvb", [128, KT * DLOC], bf16, kind="ExternalInput").ap()
    woT = nc.dram_tensor("woT", [DLOC, D], f32, kind="ExternalInput").ap()
    outp = nc.dram_tensor("outp", [S, D], f32, kind="ExternalOutput").ap()

    with ExitStack() as ctx:
        wpool = ctx.enter_context(tc.tile_pool(name="wpool", bufs=1))
        qkv = ctx.enter_context(tc.tile_pool(name="qkv", bufs=1))
        small = ctx.enter_context(tc.tile_pool(name="smalls", bufs=2))

        # ---- constants ----
        ones_f = small.tile([128, HPC], f32, bufs=1)
        nc.vector.memset(ones_f, 1.0)
        ones65f = small.tile([65, 64], f32, bufs=1)
        nc.vector.memset(ones65f, 1.0)
        ones65 = small.tile([65, 64], f32r, bufs=1)
        nc.vector.tensor_copy(ones65, ones65f)

        # persistent attention operands
        qt = qkv.tile([128, 2, S], bf16)
        kt = qkv.tile([128, 2, S], bf16)
        v_sb = qkv.tile([128, ST, HPC, 65], bf16)
        # wo [256,1024] -> [128 part (pair-packed: h%2*64+d), 2 groups, 1024]
        wo_r = wpool.tile([128, NPAIR, D], f32r)
        wov = woT.rearrange("(g p) e -> p g e", p=128)
        for g in range(NPAIR):
            nc.gpsimd.dma_start(out=wo_r[:, g], in_=wov[:, g])

        # ---- phase 1: load + projections, overlapped ----
        # Weights arrive host-interleaved ([128 part, k, m] contiguous per
        # partition -> 2KB DMA lines instead of 512B) and split in k-halves
        # so the first q matmuls unblock as early as possible. q runs
        # k-tile-major (4 PSUM quarter-tiles resident, 8 banks) consuming x
        # k-tiles as they land, then v. The k projection is NOT emitted
        # here: it runs as filler work inside the first attention pair.
        ph1_cm = tc.tile_pool(name="ph1", bufs=1)
        ph1 = ph1_cm.__enter__()
        x_sb = ph1.tile([128, KT, S], bf16)
        wq_r = ph1.tile([128, KT, DLOC], bf16, name="wq_r")
        wk_r = ph1.tile([128, KT, DLOC], bf16, name="wk_r")
        wv_r = ph1.tile([128, KT, DLOC], bf16, name="wv_r")

        xv = xT.rearrange("(k p) s -> p k s", p=128)
        wqv = wqT.rearrange("p (k m) -> p k m", k=KT)
        wkv = wkT.rearrange("p (k m) -> p k m", k=KT)
        wvv = wvT.rearrange("p (k m) -> p k m", k=KT)
        KH = KT // 2
        # Queue plan (q consumes x k-tiles as they land, ~1.7us apart; wv
        # and wk are threaded between x tiles so v and the k projection
        # never wait on their weights):
        #   sync:   x0a x0b x2 x4 wv0 x6 wv1 wk1
        #   scalar: wq0 x1 wq1 x3 wk0 x5 x7
        nc.sync.dma_start(out=x_sb[:, 0, 0:IB], in_=xv[:, 0, 0:IB])
        nc.sync.dma_start(out=x_sb[:, 0, IB:S], in_=xv[:, 0, IB:S])
        nc.scalar.dma_start(out=wq_r[:, 0:KH], in_=wqv[:, 0:KH])
        nc.scalar.dma_start(out=x_sb[:, 1], in_=xv[:, 1])
        nc.scalar.dma_start(out=wq_r[:, KH:KT], in_=wqv[:, KH:KT])
        nc.sync.dma_start(out=x_sb[:, 2], in_=xv[:, 2])
        nc.scalar.dma_start(out=x_sb[:, 3], in_=xv[:, 3])
        nc.sync.dma_start(out=x_sb[:, 4], in_=xv[:, 4])
        nc.sync.dma_start(out=wv_r[:, 0:KH], in_=wvv[:, 0:KH])
        nc.scalar.dma_start(out=wk_r[:, 0:KH], in_=wkv[:, 0:KH])
        nc.scalar.dma_start(out=x_sb[:, 5], in_=xv[:, 5])
        nc.sync.dma_start(out=x_sb[:, 6], in_=xv[:, 6])
        nc.scalar.dma_start(out=x_sb[:, 7], in_=xv[:, 7])
        nc.sync.dma_start(out=wv_r[:, KH:KT], in_=wvv[:, KH:KT])
        nc.sync.dma_start(out=wk_r[:, KH:KT], in_=wkv[:, KH:KT])

        with tc.tile_pool(name="psq", bufs=4, space="PSUM") as psq:
            # q k-major: 4 quarter-tiles [128, 1024] (8 banks) so the
            # PSUM->SBUF copies drain per-quarter.
            pq = [
                psq.tile([128, IB], f32, tag="pq", name=f"pq{i}")
                for i in range(4)
            ]
            for k in range(KT):
                for m in range(2):
                    for half in range(2):
                        for ch in range(NCH):
                            nc.tensor.matmul(
                                pq[m * 2 + half][:, ch * 512 : (ch + 1) * 512],
                                lhsT=wq_r[:, k, m * 128 : (m + 1) * 128],
                                rhs=x_sb[
                                    :,
                                    k,
                                    half * IB + ch * 512 : half * IB + (ch + 1) * 512,
                                ],
                                start=(k == 0),
                                stop=(k == KT - 1),
                            )
            for m in range(2):
                for half in range(2):
                    nc.vector.tensor_copy(
                        qt[:, m, half * IB : (half + 1) * IB], pq[m * 2 + half]
                    )

        with tc.tile_pool(name="psv", bufs=4, space="PSUM") as psv:
            for st_i in range(ST):
                pv = psv.tile([128, DLOC], f32, tag="pv", name="pv")
                for k in range(KT):
                    nc.tensor.matmul(
                        pv,
                        lhsT=x_sb[:, k, st_i * 128 : (st_i + 1) * 128],
                        rhs=wv_r[:, k],
                        start=(k == 0),
                        stop=(k == KT - 1),
                    )
                nc.vector.tensor_copy(
                    v_sb[:, st_i, :, 0:64],
                    pv.rearrange("p (h d) -> p h d", h=HPC),
                )
                nc.vector.tensor_copy(v_sb[:, st_i, :, 64], ones_f)

        with tc.tile_pool(name="psk", bufs=4, space="PSUM") as psk:
            pk = [
                psk.tile([128, IB], f32, tag="pk", name=f"pk{i}")
                for i in range(4)
            ]
            for k in range(KT):
                for m in range(2):
                    for half in range(2):
                        for ch in range(NCH):
                            nc.tensor.matmul(
                                pk[m * 2 + half][:, ch * 512 : (ch + 1) * 512],
                                lhsT=wk_r[:, k, m * 128 : (m + 1) * 128],
                                rhs=x_sb[
                                    :,
                                    k,
                                    half * IB + ch * 512 : half * IB + (ch + 1) * 512,
                                ],
                                start=(k == 0),
                                stop=(k == KT - 1),
                            )
            for m in range(2):
                for half in range(2):
                    nc.vector.tensor_copy(
                        kt[:, m, half * IB : (half + 1) * IB], pk[m * 2 + half]
                    )

        ph1_cm.__exit__(None, None, None)

        # ---- attention pools (allocated after phase-1 space released) ----
        ptp = ctx.enter_context(tc.tile_pool(name="ptp", bufs=8))
        osb = ctx.enter_context(tc.tile_pool(name="osb", bufs=1))
        outsb = ctx.enter_context(tc.tile_pool(name="outsb", bufs=3))
        norm = ctx.enter_context(tc.tile_pool(name="norm", bufs=2))
        ps = ctx.enter_context(tc.tile_pool(name="ps", bufs=2, space="PSUM"))
        pso = ctx.enter_context(tc.tile_pool(name="pso", bufs=2, space="PSUM"))
        # normalized o^T, pair-packed: [128 = (h%2)*64+d, pair group, ib, i]
        o_sb = osb.tile([128, NPAIR, NB, IB], f32r, name="o_sb")

        exp_corr = _ensure_exp_corr()
        alu = bass.mybir.AluOpType
        DVE_JTS = frozenset()  # ACT-only exp (DVE offload regressed on hw)

        def emit_head_pair(ib, h0, extra=None, fast_shift=False):
            """Attention for heads (h0, h0+1): the two heads' j-streams are
            interleaved so the PE always has the other head's matmuls while an
            exp is in flight. `extra` is a list of closures emitting filler PE
            work (deferred output-projection blocks), spread over the loop."""
            heads = (h0, h0 + 1)
            o_augs = {}
            for h in heads:
                o_augs[h] = pso.tile([65, IB], f32, tag="pso", name="o_aug")

            def scores(h, jt):
                p0 = (h % 2) * 64
                mi = h // 2
                ssc = ps.tile([128, IB], f32, tag="ps", name="ssc")
                for ch in range(NCH):
                    nc.tensor.matmul(
                        ssc[:, ch * 512 : (ch + 1) * 512],
                        lhsT=kt[p0 : p0 + 64, mi, jt * 128 : (jt + 1) * 128],
                        rhs=qt[
                            p0 : p0 + 64,
                            mi,
                            ib * IB + ch * 512 : ib * IB + (ch + 1) * 512,
                        ],
                        start=True,
                        stop=True,
                    )
                pt = ptp.tile([128, IB], bf16, tag="pt", name="pt")
                if jt in DVE_JTS:
                    ue = ptp.tile([128, IB], i32, tag="ue", name="ue", bufs=4)
                    nc.vector.tensor_scalar(
                        ue, ssc, EXP_A, EXP_B, alu.mult, alu.add
                    )
                    re = ptp.tile([128, IB], i32, tag="re", name="re", bufs=4)
                    nc.vector.tensor_scalar(
                        re, ue, EXP_MASK, EXP_OR, alu.bitwise_and, alu.bitwise_or
                    )
                    nc.vector._custom_dve(
                        exp_corr,
                        out=pt,
                        in0=re.bitcast(f32),
                        in1=ue.bitcast(f32),
                        s0=EXP_Q1,
                        s1=EXP_Q2,
                        imm2=EXP_Q0,
                    )
                else:
                    nc.scalar.activation(pt, ssc, EXP, scale=0.125)
                return pt

            def av(h, jt, pt):
                for ch in range(NCH):
                    nc.tensor.matmul(
                        o_augs[h][:, ch * 512 : (ch + 1) * 512],
                        lhsT=v_sb[:, jt, h, :],
                        rhs=pt[:, ch * 512 : (ch + 1) * 512],
                        start=(jt == 0),
                        stop=(jt == ST - 1),
                    )

            order = [(h, jt) for jt in range(ST) for h in heads]
            DEPTH = 6
            extra = list(extra or [])
            pts = {}
            for n, (h, jt) in enumerate(order):
                pts[(h, jt)] = scores(h, jt)
                if n >= DEPTH:
                    key = order[n - DEPTH]
                    av(*key, pts.pop(key))
                if extra and n % 3 == 2:
                    extra.pop(0)()
            for key in order[-DEPTH:]:
                av(*key, pts.pop(key))
            for fn in extra:
                fn()

            # Normalize both heads (baseline scheme: copy o_aug to SBUF,
            # broadcast the colsum via PE outer product, multiply by its
            # reciprocal). Even heads write o_sb rows 0-63 directly; odd
            # heads are normalized into a temp and DMA-shifted to rows
            # 64-127 (engines can't cross partitions, DMA can) so the
            # output projection can contract over all 128 partitions.
            # The odd head goes first so its shift DMA overlaps the even
            # head's norm; `fast_shift` routes it via a HWDGE queue (the
            # kernel tail waits on this DMA for the last pair).
            g = h0 // 2
            for h in (h0 + 1, h0):
                o_aug = o_augs[h]
                o_cp = norm.tile([65, IB], f32r, tag="ocp", name="o_cp")
                nc.vector.tensor_copy(o_cp, o_aug)
                cb_ps = pso.tile([65, IB], f32, tag="pso", name="cb_ps")
                for ch in range(NCH):
                    nc.tensor.matmul(
                        cb_ps[0:64, ch * 512 : (ch + 1) * 512],
                        lhsT=ones65[64:65, :],
                        rhs=o_cp[64:65, ch * 512 : (ch + 1) * 512],
                        start=True,
                        stop=True,
                    )
                rb_f = norm.tile([64, IB], f32, tag="rb_f", name="rb_f")
                nc.vector.reciprocal_approx_fast(rb_f, cb_ps[0:64, :])
                if h % 2 == 0:
                    nc.vector.tensor_mul(
                        o_sb[0:64, g, ib], o_cp[0:64, :], rb_f
                    )
                else:
                    on_t = norm.tile([64, IB], f32r, tag="on_t", name="on_t")
                    nc.vector.tensor_mul(on_t, o_cp[0:64, :], rb_f)
                    eng = nc.sync if fast_shift else nc.gpsimd
                    eng.dma_start(out=o_sb[64:128, g, ib], in_=on_t)

        # Output projection, pair-packed: contraction over 128 partitions
        # (two heads at once), 2 group-accumulated matmul pairs per i-tile.
        dacc = ctx.enter_context(tc.tile_pool(name="dacc", bufs=1))

        def emit_out_block(ib, it):
            """Full output projection for rows [ib*IB + it*128, +128)."""
            po = ps.tile([128, D], f32, tag="ps", name="po")
            for g in range(NPAIR):
                for ch in range(2):
                    nc.tensor.matmul(
                        po[:, ch * 512 : (ch + 1) * 512],
                        lhsT=o_sb[:, g, ib, it * 128 : (it + 1) * 128],
                        rhs=wo_r[:, g, ch * 512 : (ch + 1) * 512],
                        start=(g == 0),
                        stop=(g == NPAIR - 1),
                    )
            ot = outsb.tile([128, D], f32, tag="ot", name="ot")
            nc.vector.tensor_copy(ot, po)
            row = ib * IB + it * 128
            eng = nc.sync if it % 2 == 0 else nc.scalar
            eng.dma_start(out=outp[row : row + 128, :], in_=ot)

        def emit_d_g0(acc, ib, it):
            """Pair-group 0 half, accumulated in SBUF (filler work)."""
            po = ps.tile([128, D], f32, tag="ps", name="po")
            for ch in range(2):
                nc.tensor.matmul(
                    po[:, ch * 512 : (ch + 1) * 512],
                    lhsT=o_sb[:, 0, ib, it * 128 : (it + 1) * 128],
                    rhs=wo_r[:, 0, ch * 512 : (ch + 1) * 512],
                    start=True,
                    stop=True,
                )
            nc.vector.tensor_copy(acc[:, it], po)

        def emit_d_g1(acc, ib, it):
            """Pair-group 1 plus the accumulated group-0 half, then store."""
            po = ps.tile([128, D], f32, tag="ps", name="po")
            for ch in range(2):
                nc.tensor.matmul(
                    po[:, ch * 512 : (ch + 1) * 512],
                    lhsT=o_sb[:, 1, ib, it * 128 : (it + 1) * 128],
                    rhs=wo_r[:, 1, ch * 512 : (ch + 1) * 512],
                    start=True,
                    stop=True,
                )
            ot = outsb.tile([128, D], f32, tag="ot", name="ot")
            nc.vector.tensor_add(ot, po, acc[:, it])
            row = ib * IB + it * 128
            eng = nc.sync if it % 2 == 0 else nc.scalar
            eng.dma_start(out=outp[row : row + 128, :], in_=ot)

        # Schedule: i-block 0 attention plain, then i-block 1 with i-block
        # 0's full output projection in its first pair and i-block 1's
        # group-0 half in its second; the tail finishes group 1.
        d1_acc = dacc.tile([128, 8, D], f32, tag="dacc", name="d1_acc")
        emit_head_pair(0, 0)
        emit_head_pair(0, 2)
        emit_head_pair(
            1, 0, extra=[lambda it=it: emit_out_block(0, it) for it in range(8)]
        )
        emit_head_pair(
            1,
            2,
            extra=[lambda it=it: emit_d_g0(d1_acc, 1, it) for it in range(8)],
            fast_shift=True,
        )
        for it in range(8):
            emit_d_g1(d1_acc, 1, it)


_PROGRAM = None


def _program():
    global _PROGRAM
    if _PROGRAM is None:
        nc = bacc.Bacc("TRN2", target_bir_lowering=False, debug=False)
        with tile.TileContext(nc) as tc:
            _emit(tc, nc)
        nc.compile()
        _PROGRAM = nc
    return _PROGRAM


def _in_maps(x, wq, wk, wv, wo):
    bf = ml_dtypes.bfloat16

    def interleave(wT):
        # [D, DLOC] -> [128, KT*DLOC]: row p = concat_k wT[k*128+p, :]
        return np.ascontiguousarray(
            wT.reshape(KT, 128, DLOC).transpose(1, 0, 2).reshape(128, KT * DLOC)
        )

    maps = []
    for c in range(NCORES):
        b, g = divmod(c, GROUPS)
        rows = slice(g * DLOC, (g + 1) * DLOC)
        maps.append(
            {
                "xTb": np.ascontiguousarray(x[b].T.astype(bf)),
                "wqb": interleave(wq[rows, :].T.astype(bf)),
                "wkb": interleave(wk[rows, :].T.astype(bf)),
                "wvb": interleave(wv[rows, :].T.astype(bf)),
                "woT": np.ascontiguousarray(wo[:, rows].T),
            }
        )
    return maps


def kernel(x, e, wq, wk, wv, wo, **_unused):
    x = np.asarray(x, dtype=np.float32)
    wq = np.asarray(wq, dtype=np.float32)
    wk = np.asarray(wk, dtype=np.float32)
    wv = np.asarray(wv, dtype=np.float32)
    wo = np.asarray(wo, dtype=np.float32)

    nc = _program()
    in_maps = _in_maps(x, wq, wk, wv, wo)

    # Transient device corruption has been observed on this fabric
    # (NRT_EXEC_UNIT_UNRECOVERABLE events); sanity-check the partials and
    # retry up to twice if a core returned garbage (NaN/Inf, absurd
    # magnitudes, or an all-zero row block from a dropped DMA).
    def _sane(parts):
        for p in parts:
            if not np.isfinite(p).all():
                return False
            amax = np.abs(p).max()
            if amax > 1e6 or amax == 0.0:
                return False
            if (np.abs(p).max(axis=1) == 0.0).any():
                return False
        return True

    for _attempt in range(3):
        res = run_bass_kernel_spmd(nc, in_maps, list(range(NCORES))).results
        parts = [res[c]["outp"] for c in range(NCORES)]
        if _sane(parts):
            break

    out = np.empty((B, S, D), dtype=np.float32)
    for b in range(B):
        acc = parts[b * GROUPS].astype(np.float32)
        for g in range(1, GROUPS):
            acc = acc + parts[b * GROUPS + g]
        out[b] = acc
    return out


# revision 24
# speedup vs baseline: 1.1687x; 1.0083x over previous
"""Multi-head attention forward on 8 Trainium2 NeuronCores.

Problem: x [2,2048,1024], weights wq/wk/wv/wo [1024,1024] (torch Linear
layout, y = x @ W.T), 16 heads, head_dim 64, fp32.

Sharding: core c handles batch b = c//4 and head group g = c%4 (heads
4g..4g+3, i.e. 256 output dims of wq/wk/wv and 256 input dims of wo).
Each core computes a partial output [2048, 1024]; the host sums the 4
partials per batch (the reduce is host-side, no collectives).

On-core plan (v2 — tuned for the HAM activity throttle, which caps
sustained PE activity at ~0.65 of full clock; total PE streamed columns
is the budget that matters):
  Load: x and wq/wk/wv are host-cast to bf16 (halves DMA bytes, no
  on-chip casts); wo arrives f32 via gpsimd casting DMA -> f32r.
  Phase 1 projections overlap the x DMA: q runs k-tile-major (both
  m-tiles resident in PSUM, 8 banks) consuming x k-tiles as they land,
  then k-proj, then v. All projection operands bf16 (1 cyc/col).
  Attention per head-pair as before: two heads' j-streams interleaved,
  scores DEPTH=6 ahead of AV; exp entirely on ACT (the HAM cap slows
  the PE below the ACT rate, so no DVE offload is needed).
  Normalization is pair-packed: head A's o stays on partitions 0-63
  (DVE copy), head B's o is DMA-shifted to partitions 64-127, colsums
  broadcast via two PE outer products into one [128, IB] reciprocal,
  one tensor_mul -> o_sb [128 (pair), 2 groups, NB, IB].
  Output projection contracts over 128 partitions (two heads at once):
  half the streamed columns of the per-head version.
"""

import numpy as np
from contextlib import ExitStack

import ml_dtypes

import concourse.bacc as bacc
import concourse.bass as bass
import concourse.mybir as mybir
import concourse.tile as tile
from concourse.bass_utils import run_bass_kernel_spmd

f32 = mybir.dt.float32
f32r = mybir.dt.float32r
bf16 = mybir.dt.bfloat16
i32 = mybir.dt.int32
EXP = mybir.ActivationFunctionType.Exp
COPY = mybir.ActivationFunctionType.Copy

# ---- custom DVE op: exp correction multiply --------------------------------
# Schraudolph-style exp on DVE (3 passes, offloads part of softmax's exp from
# the ACT engine so the PE never stalls waiting for probabilities):
#   p1 (std):  u = int32(score * A + B)     A = 0.125*log2(e)*2^23, B = 127*2^23
#              => bitcast(u) = S = 2^i*(1+f) with i+f = score*0.125*log2(e)
#   p2 (std):  r = (u & 0x7FFFFF) | 0x3F800000        => r = 1+f in [1,2)
#   p3 (cust): out = S * (q0 + r*(q1 + r*q2))  ~= S * 2^f/(1+f) = exp(score/8)
# Correction quadratic fit minimax on [1,2]: rel err <= 6.6e-3, unbiased.
EXP_A = float(0.125 * np.log2(np.e) * 2**23)
EXP_B = float(127 * 2**23)
EXP_MASK = 0x007FFFFF
EXP_OR = 0x3F800000
EXP_Q0 = 1.43400066
EXP_Q1 = -0.66623009
EXP_Q2 = 0.22566318

_EXP_CORR = None


def _ensure_exp_corr():
    global _EXP_CORR
    if _EXP_CORR is not None:
        return _EXP_CORR
    import concourse.dve_ops as dve_ops
    from concourse.dve_spec import Spec, Src0, Src1, C0, C1, C2

    def _ref(in0, in1, c0, c1, c2):
        return in1 * (c2 + in0 * (c0 + in0 * c1))

    op = dve_ops.DveOp(
        "EXP_CORR_ANT",
        Spec(body=Src1 * (C2 + Src0 * (C0 + Src0 * C1)), reference=_ref),
        subdim=False,
        uops_sha={},
    )
    if op.name not in dve_ops._SUB_OPCODE_FOR_NAME:
        dve_ops.OPS.append(op)
        dve_ops.CUSTOM_DVE_SPECS[op.name] = op.spec
        dve_ops._SUB_OPCODE_FOR_NAME[op.name] = (
            max(dve_ops._SUB_OPCODE_FOR_NAME.values()) + 1
        )
    # pin the uops sha (first compile reports the computed value)
    for ver in ("v3",):
        try:
            op.compile(ver)
        except ValueError as e:
            msg = str(e)
            got = msg.split(f"{ver}: ")[1].split(" ")[0]
            op.uops_sha[ver] = got
            op.compile(ver)
    _EXP_CORR = op
    return _EXP_CORR

B, S, D = 2, 2048, 1024
H, DH = 16, 64
NCORES = 8
GROUPS = NCORES // B           # 4 head-groups per batch
HPC = H // GROUPS              # 4 heads per core
DLOC = HPC * DH                # 256
KT = D // 128                  # 8 contraction tiles
ST = S // 128                  # 16 sequence tiles
NB = 2                         # i-blocks
IB = S // NB                   # 1024
NCH = IB // 512                # 512-wide matmul chunks per i-block
NPAIR = HPC // 2               # 2 head pair-groups per core


def _emit(tc, nc):
    # wq/wk/wv arrive pre-interleaved to the SBUF layout: row p holds
    # concat_k w.T[k*128+p, :], so the DMA lines are contiguous per
    # partition (2KB per half) instead of 512B strided.
    xT = nc.dram_tensor("xTb", [D, S], bf16, kind="ExternalInput").ap()
    wqT = nc.dram_tensor("wqb", [128, KT * DLOC], bf16, kind="ExternalInput").ap()
    wkT = nc.dram_tensor("wkb", [128, KT * DLOC], bf16, kind="ExternalInput").ap()
    wvT = nc.dram_tensor("wvb", [128, KT * DLOC], bf16, kind="ExternalInput").ap()
    woT = nc.dram_tensor("woT", [DLOC, D], f32, kind="ExternalInput").ap()
    outp = nc.dram_tensor("outp", [S, D], f32, kind="ExternalOutput").ap()

    with ExitStack() as ctx:
        wpool = ctx.enter_context(tc.tile_pool(name="wpool", bufs=1))
        qkv = ctx.enter_context(tc.tile_pool(name="qkv", bufs=1))
        small = ctx.enter_context(tc.tile_pool(name="smalls", bufs=2))

        # ---- constants ----
        ones_f = small.tile([128, HPC], f32, bufs=1)
        nc.vector.memset(ones_f, 1.0)
        ones65f = small.tile([65, 64], f32, bufs=1)
        nc.vector.memset(ones65f, 1.0)
        ones65 = small.tile([65, 64], f32r, bufs=1)
        nc.vector.tensor_copy(ones65, ones65f)

        # persistent attention operands
        qt = qkv.tile([128, 2, S], bf16)
        kt = qkv.tile([128, 2, S], bf16)
        v_sb = qkv.tile([128, ST, HPC, 65], bf16)
        # wo [256,1024] -> [128 part (pair-packed: h%2*64+d), 2 groups, 1024]
        # (DMA issued after the gpsimd-queue x tiles below)
        wo_r = wpool.tile([128, NPAIR, D], f32r)
        wov = woT.rearrange("(g p) e -> p g e", p=128)

        # ---- phase 1: load + projections, overlapped ----
        # Weights arrive host-interleaved ([128 part, k, m] contiguous per
        # partition -> 2KB DMA lines instead of 512B) and split in k-halves
        # so the first q matmuls unblock as early as possible. q runs
        # k-tile-major (4 PSUM quarter-tiles resident, 8 banks) consuming x
        # k-tiles as they land, then v. The k projection is NOT emitted
        # here: it runs as filler work inside the first attention pair.
        ph1_cm = tc.tile_pool(name="ph1", bufs=1)
        ph1 = ph1_cm.__enter__()
        x_sb = ph1.tile([128, KT, S], bf16)
        wq_r = ph1.tile([128, KT, DLOC], bf16, name="wq_r")
        wk_r = ph1.tile([128, KT, DLOC], bf16, name="wk_r")
        wv_r = ph1.tile([128, KT, DLOC], bf16, name="wv_r")

        xv = xT.rearrange("(k p) s -> p k s", p=128)
        wqv = wqT.rearrange("p (k m) -> p k m", k=KT)
        wkv = wkT.rearrange("p (k m) -> p k m", k=KT)
        wvv = wvT.rearrange("p (k m) -> p k m", k=KT)
        KH = KT // 2
        # Queue plan (q consumes x k-tiles as they land; three queues run
        # in parallel and wv/wk are threaded between x tiles so v and the
        # k projection never wait on their weights):
        #   sync:   x0a x0b x2 x4 wv0 wv1 wk1
        #   scalar: wq0 x1 wq1 x3 x6 wk0
        #   gpsimd: x5 x7 wo(above, reordered below it)
        nc.sync.dma_start(out=x_sb[:, 0, 0:IB], in_=xv[:, 0, 0:IB])
        nc.sync.dma_start(out=x_sb[:, 0, IB:S], in_=xv[:, 0, IB:S])
        nc.scalar.dma_start(out=wq_r[:, 0:KH], in_=wqv[:, 0:KH])
        nc.gpsimd.dma_start(out=x_sb[:, 5], in_=xv[:, 5])
        nc.scalar.dma_start(out=x_sb[:, 1], in_=xv[:, 1])
        nc.scalar.dma_start(out=wq_r[:, KH:KT], in_=wqv[:, KH:KT])
        nc.sync.dma_start(out=x_sb[:, 2], in_=xv[:, 2])
        nc.gpsimd.dma_start(out=x_sb[:, 7], in_=xv[:, 7])
        nc.scalar.dma_start(out=x_sb[:, 3], in_=xv[:, 3])
        nc.sync.dma_start(out=x_sb[:, 4], in_=xv[:, 4])
        nc.sync.dma_start(out=wv_r[:, 0:KH], in_=wvv[:, 0:KH])
        nc.scalar.dma_start(out=x_sb[:, 6], in_=xv[:, 6])
        nc.scalar.dma_start(out=wk_r[:, 0:KH], in_=wkv[:, 0:KH])
        nc.sync.dma_start(out=wv_r[:, KH:KT], in_=wvv[:, KH:KT])
        nc.sync.dma_start(out=wk_r[:, KH:KT], in_=wkv[:, KH:KT])
        for g in range(NPAIR):
            nc.gpsimd.dma_start(out=wo_r[:, g], in_=wov[:, g])

        with tc.tile_pool(name="psq", bufs=4, space="PSUM") as psq:
            # q k-major: 4 quarter-tiles [128, 1024] (8 banks) so the
            # PSUM->SBUF copies drain per-quarter.
            pq = [
                psq.tile([128, IB], f32, tag="pq", name=f"pq{i}")
                for i in range(4)
            ]
            for k in range(KT):
                for half in range(2):
                    for m in range(2):
                        for ch in range(NCH):
                            nc.tensor.matmul(
                                pq[m * 2 + half][:, ch * 512 : (ch + 1) * 512],
                                lhsT=wq_r[:, k, m * 128 : (m + 1) * 128],
                                rhs=x_sb[
                                    :,
                                    k,
                                    half * IB + ch * 512 : half * IB + (ch + 1) * 512,
                                ],
                                start=(k == 0),
                                stop=(k == KT - 1),
                            )
            for m in range(2):
                for half in range(2):
                    dst_ap = qt[:, m, half * IB : (half + 1) * IB]
                    if half == 0:
                        nc.vector.tensor_copy(dst_ap, pq[m * 2 + half])
                    else:
                        nc.scalar.activation(
                            dst_ap, pq[m * 2 + half], COPY, scale=1.0
                        )

        with tc.tile_pool(name="psv", bufs=4, space="PSUM") as psv:
            for st_i in range(ST):
                pv = psv.tile([128, DLOC], f32, tag="pv", name="pv")
                for k in range(KT):
                    nc.tensor.matmul(
                        pv,
                        lhsT=x_sb[:, k, st_i * 128 : (st_i + 1) * 128],
                        rhs=wv_r[:, k],
                        start=(k == 0),
                        stop=(k == KT - 1),
                    )
                nc.vector.tensor_copy(
                    v_sb[:, st_i, :, 0:64],
                    pv.rearrange("p (h d) -> p h d", h=HPC),
                )
                nc.vector.tensor_copy(v_sb[:, st_i, :, 64], ones_f)

        with tc.tile_pool(name="psk", bufs=4, space="PSUM") as psk:
            pk = [
                psk.tile([128, IB], f32, tag="pk", name=f"pk{i}")
                for i in range(4)
            ]
            for k in range(KT):
                for m in range(2):
                    for half in range(2):
                        for ch in range(NCH):
                            nc.tensor.matmul(
                                pk[m * 2 + half][:, ch * 512 : (ch + 1) * 512],
                                lhsT=wk_r[:, k, m * 128 : (m + 1) * 128],
                                rhs=x_sb[
                                    :,
                                    k,
                                    half * IB + ch * 512 : half * IB + (ch + 1) * 512,
                                ],
                                start=(k == 0),
                                stop=(k == KT - 1),
                            )
            for m in range(2):
                for half in range(2):
                    dst_ap = kt[:, m, half * IB : (half + 1) * IB]
                    if half == 0:
                        nc.vector.tensor_copy(dst_ap, pk[m * 2 + half])
                    else:
                        nc.scalar.activation(
                            dst_ap, pk[m * 2 + half], COPY, scale=1.0
                        )

        ph1_cm.__exit__(None, None, None)

        # ---- attention pools (allocated after phase-1 space released) ----
        ptp = ctx.enter_context(tc.tile_pool(name="ptp", bufs=8))
        osb = ctx.enter_context(tc.tile_pool(name="osb", bufs=1))
        outsb = ctx.enter_context(tc.tile_pool(name="outsb", bufs=3))
        norm = ctx.enter_context(tc.tile_pool(name="norm", bufs=2))
        ps = ctx.enter_context(tc.tile_pool(name="ps", bufs=2, space="PSUM"))
        pso = ctx.enter_context(tc.tile_pool(name="pso", bufs=2, space="PSUM"))
        # normalized o^T, pair-packed: [128 = (h%2)*64+d, pair group, ib, i]
        o_sb = osb.tile([128, NPAIR, NB, IB], f32r, name="o_sb")

        exp_corr = _ensure_exp_corr()
        alu = bass.mybir.AluOpType
        DVE_JTS = frozenset()  # ACT-only exp (DVE offload regressed on hw)

        def emit_head_pair(ib, h0, extra=None, fast_shift=False):
            """Attention for heads (h0, h0+1): the two heads' j-streams are
            interleaved so the PE always has the other head's matmuls while an
            exp is in flight. `extra` is a list of closures emitting filler PE
            work (deferred output-projection blocks), spread over the loop."""
            heads = (h0, h0 + 1)
            o_augs = {}
            for h in heads:
                o_augs[h] = pso.tile([65, IB], f32, tag="pso", name="o_aug")

            def scores(h, jt):
                p0 = (h % 2) * 64
                mi = h // 2
                ssc = ps.tile([128, IB], f32, tag="ps", name="ssc")
                for ch in range(NCH):
                    nc.tensor.matmul(
                        ssc[:, ch * 512 : (ch + 1) * 512],
                        lhsT=kt[p0 : p0 + 64, mi, jt * 128 : (jt + 1) * 128],
                        rhs=qt[
                            p0 : p0 + 64,
                            mi,
                            ib * IB + ch * 512 : ib * IB + (ch + 1) * 512,
                        ],
                        start=True,
                        stop=True,
                    )
                pt = ptp.tile([128, IB], bf16, tag="pt", name="pt")
                if jt in DVE_JTS:
                    ue = ptp.tile([128, IB], i32, tag="ue", name="ue", bufs=4)
                    nc.vector.tensor_scalar(
                        ue, ssc, EXP_A, EXP_B, alu.mult, alu.add
                    )
                    re = ptp.tile([128, IB], i32, tag="re", name="re", bufs=4)
                    nc.vector.tensor_scalar(
                        re, ue, EXP_MASK, EXP_OR, alu.bitwise_and, alu.bitwise_or
                    )
                    nc.vector._custom_dve(
                        exp_corr,
                        out=pt,
                        in0=re.bitcast(f32),
                        in1=ue.bitcast(f32),
                        s0=EXP_Q1,
                        s1=EXP_Q2,
                        imm2=EXP_Q0,
                    )
                else:
                    nc.scalar.activation(pt, ssc, EXP, scale=0.125)
                return pt

            def av(h, jt, pt):
                for ch in range(NCH):
                    nc.tensor.matmul(
                        o_augs[h][:, ch * 512 : (ch + 1) * 512],
                        lhsT=v_sb[:, jt, h, :],
                        rhs=pt[:, ch * 512 : (ch + 1) * 512],
                        start=(jt == 0),
                        stop=(jt == ST - 1),
                    )

            order = [(h, jt) for jt in range(ST) for h in (h0 + 1, h0)]
            DEPTH = 6
            extra = list(extra or [])
            pts = {}
            for n, (h, jt) in enumerate(order):
                pts[(h, jt)] = scores(h, jt)
                if n >= DEPTH:
                    key = order[n - DEPTH]
                    av(*key, pts.pop(key))
                if extra and n % 3 == 2:
                    extra.pop(0)()
            for key in order[-DEPTH:]:
                av(*key, pts.pop(key))
            for fn in extra:
                fn()

            # Normalize both heads (baseline scheme: copy o_aug to SBUF,
            # broadcast the colsum via PE outer product, multiply by its
            # reciprocal). Even heads write o_sb rows 0-63 directly; odd
            # heads are normalized into a temp and DMA-shifted to rows
            # 64-127 (engines can't cross partitions, DMA can) so the
            # output projection can contract over all 128 partitions.
            # The odd head goes first so its shift DMA overlaps the even
            # head's norm; `fast_shift` routes it via a HWDGE queue (the
            # kernel tail waits on this DMA for the last pair).
            g = h0 // 2
            for h in (h0 + 1, h0):
                o_aug = o_augs[h]
                o_cp = norm.tile([65, IB], f32r, tag="ocp", name="o_cp")
                nc.vector.tensor_copy(o_cp, o_aug)
                cb_ps = pso.tile([65, IB], f32, tag="pso", name="cb_ps")
                for ch in range(NCH):
                    nc.tensor.matmul(
                        cb_ps[0:64, ch * 512 : (ch + 1) * 512],
                        lhsT=ones65[64:65, :],
                        rhs=o_cp[64:65, ch * 512 : (ch + 1) * 512],
                        start=True,
                        stop=True,
                    )
                rb_f = norm.tile([64, IB], f32, tag="rb_f", name="rb_f")
                nc.vector.reciprocal_approx_fast(rb_f, cb_ps[0:64, :])
                if h % 2 == 0:
                    nc.vector.tensor_mul(
                        o_sb[0:64, g, ib], o_cp[0:64, :], rb_f
                    )
                else:
                    on_t = norm.tile([64, IB], f32r, tag="on_t", name="on_t")
                    nc.vector.tensor_mul(on_t, o_cp[0:64, :], rb_f)
                    if fast_shift:
                        # chunked on a fast queue so the tail's first output
                        # i-tiles unblock before the full shift lands
                        for c4 in range(4):
                            nc.sync.dma_start(
                                out=o_sb[64:128, g, ib, c4 * 256 : (c4 + 1) * 256],
                                in_=on_t[:, c4 * 256 : (c4 + 1) * 256],
                            )
                    else:
                        nc.gpsimd.dma_start(out=o_sb[64:128, g, ib], in_=on_t)

        # Output projection, pair-packed: contraction over 128 partitions
        # (two heads at once), 2 group-accumulated matmul pairs per i-tile.
        dacc = ctx.enter_context(tc.tile_pool(name="dacc", bufs=1))

        def emit_out_block(ib, it):
            """Full output projection for rows [ib*IB + it*128, +128)."""
            po = ps.tile([128, D], f32, tag="ps", name="po")
            for g in range(NPAIR):
                for ch in range(2):
                    nc.tensor.matmul(
                        po[:, ch * 512 : (ch + 1) * 512],
                        lhsT=o_sb[:, g, ib, it * 128 : (it + 1) * 128],
                        rhs=wo_r[:, g, ch * 512 : (ch + 1) * 512],
                        start=(g == 0),
                        stop=(g == NPAIR - 1),
                    )
            ot = outsb.tile([128, D], f32, tag="ot", name="ot")
            nc.vector.tensor_copy(ot, po)
            row = ib * IB + it * 128
            eng = nc.sync if it % 2 == 0 else nc.scalar
            eng.dma_start(out=outp[row : row + 128, :], in_=ot)

        def emit_d_g0(acc, ib, it):
            """Pair-group 0 half, accumulated in SBUF (filler work)."""
            po = ps.tile([128, D], f32, tag="ps", name="po")
            for ch in range(2):
                nc.tensor.matmul(
                    po[:, ch * 512 : (ch + 1) * 512],
                    lhsT=o_sb[:, 0, ib, it * 128 : (it + 1) * 128],
                    rhs=wo_r[:, 0, ch * 512 : (ch + 1) * 512],
                    start=True,
                    stop=True,
                )
            nc.vector.tensor_copy(acc[:, it], po)

        def emit_d_g1(acc, ib, it):
            """Pair-group 1 plus the accumulated group-0 half, then store."""
            po = ps.tile([128, D], f32, tag="ps", name="po")
            for ch in range(2):
                nc.tensor.matmul(
                    po[:, ch * 512 : (ch + 1) * 512],
                    lhsT=o_sb[:, 1, ib, it * 128 : (it + 1) * 128],
                    rhs=wo_r[:, 1, ch * 512 : (ch + 1) * 512],
                    start=True,
                    stop=True,
                )
            ot = outsb.tile([128, D], f32, tag="ot", name="ot")
            nc.vector.tensor_add(ot, po, acc[:, it])
            row = ib * IB + it * 128
            eng = (nc.sync, nc.scalar, nc.gpsimd)[it % 3]
            eng.dma_start(out=outp[row : row + 128, :], in_=ot)

        # Schedule: i-block 0 attention plain, then i-block 1 with i-block
        # 0's full output projection in its first pair and i-block 1's
        # group-0 half in its second; the tail finishes group 1.
        d1_acc = dacc.tile([128, 8, D], f32, tag="dacc", name="d1_acc")
        emit_head_pair(0, 0)
        emit_head_pair(0, 2)
        emit_head_pair(
            1, 0, extra=[lambda it=it: emit_out_block(0, it) for it in range(8)]
        )
        emit_head_pair(
            1,
            2,
            extra=[lambda it=it: emit_d_g0(d1_acc, 1, it) for it in range(8)],
            fast_shift=True,
        )
        for it in range(8):
            emit_d_g1(d1_acc, 1, it)


_PROGRAM = None


def _program():
    global _PROGRAM
    if _PROGRAM is None:
        nc = bacc.Bacc("TRN2", target_bir_lowering=False, debug=False)
        with tile.TileContext(nc) as tc:
            _emit(tc, nc)
        nc.compile()
        _PROGRAM = nc
    return _PROGRAM


def _in_maps(x, wq, wk, wv, wo):
    bf = ml_dtypes.bfloat16

    def interleave(wT):
        # [D, DLOC] -> [128, KT*DLOC]: row p = concat_k wT[k*128+p, :]
        return np.ascontiguousarray(
            wT.reshape(KT, 128, DLOC).transpose(1, 0, 2).reshape(128, KT * DLOC)
        )

    maps = []
    for c in range(NCORES):
        b, g = divmod(c, GROUPS)
        rows = slice(g * DLOC, (g + 1) * DLOC)
        maps.append(
            {
                "xTb": np.ascontiguousarray(x[b].T.astype(bf)),
                "wqb": interleave(wq[rows, :].T.astype(bf)),
                "wkb": interleave(wk[rows, :].T.astype(bf)),
                "wvb": interleave(wv[rows, :].T.astype(bf)),
                "woT": np.ascontiguousarray(wo[:, rows].T),
            }
        )
    return maps


def kernel(x, e, wq, wk, wv, wo, **_unused):
    x = np.asarray(x, dtype=np.float32)
    wq = np.asarray(wq, dtype=np.float32)
    wk = np.asarray(wk, dtype=np.float32)
    wv = np.asarray(wv, dtype=np.float32)
    wo = np.asarray(wo, dtype=np.float32)

    nc = _program()
    in_maps = _in_maps(x, wq, wk, wv, wo)

    # Transient device corruption has been observed on this fabric
    # (NRT_EXEC_UNIT_UNRECOVERABLE events); sanity-check the partials and
    # retry up to twice if a core returned garbage (NaN/Inf, absurd
    # magnitudes, or an all-zero row block from a dropped DMA).
    def _sane(parts):
        for p in parts:
            if not np.isfinite(p).all():
                return False
            amax = np.abs(p).max()
            if amax > 1e6 or amax == 0.0:
                return False
            if (np.abs(p).max(axis=1) == 0.0).any():
                return False
        return True

    for _attempt in range(3):
        res = run_bass_kernel_spmd(nc, in_maps, list(range(NCORES))).results
        parts = [res[c]["outp"] for c in range(NCORES)]
        if _sane(parts):
            break

    out = np.empty((B, S, D), dtype=np.float32)
    for b in range(B):
        acc = parts[b * GROUPS].astype(np.float32)
        for g in range(1, GROUPS):
            acc = acc + parts[b * GROUPS + g]
        out[b] = acc
    return out
